# revision 30
# baseline (speedup 1.0000x reference)
"""Trainium2 Bass kernel for nn_LocalSwarmAggregator (sliding-window causal MHA).

Reference computation (fp32):
    q,k,v = x@Wq+bq, x@Wk+bk, x@Wv+bv          # [B,N,D] -> per-head [B,H,N,64]
    logits = q k^T / 8 + band_mask              # causal + 256-window
    out = softmax(logits) v                     # [B,H,N,64]
    y = concat_heads(out) @ Wo + bo             # [B,N,D]

Sharding over 8 cores: core c handles batch c//4 and heads 4*(c%4)..4*(c%4)+3
(tensor-parallel on the head dim of Wq/Wk/Wv and the row dim of Wo).  Each
core computes a partial y for its batch; the host sums the 4 partials per
batch and adds bo.  No cross-device communication.

v2 layout (all matmuls bf16, single software-pipelined pass):
  - host supplies x^T pre-arranged [128, D/128, N] so no on-chip transposes
    are needed for the QKV projections (contract over D).
  - q^T,k^T [128, pair, N]: weights stationary; v natural (keys on
    partitions) computed with x^T chunks stationary, +bv folded in via a
    rank-1 ones x bv broadcast added during the PSUM->SBUF copy; augmented
    with a ones column so AV yields softmax denominators for free.
  - S^T tiles [128 keys, span<=384 queries] per (kt, pair), both heads in
    one PSUM tile; P^T = exp(S^T/8) * band01 (logits are O(6): exp safe).
  - AV transposed: out[q, 65] accumulated over the <=3 key tiles covering
    the 128-query tile, with P^T chunks *stationary* and v_aug moving (65
    cols/matmul, half the PE cost of moving-query AV and no PSUM zero-fill
    dummy).  Denominator lands in column 64 -> per-partition reciprocal +
    scale, then one PE transpose puts normalized heads back on partitions
    for the output projection.
  - out-proj per 128-query tile (contract head pairs K=128), emitted one
    tile behind AV so the PE never waits on the normalize/transpose chain.
"""

import os
from contextlib import ExitStack

import numpy as np

import concourse.bass as bass
import concourse.mybir as mybir
import concourse.tile as tile
from concourse import bacc
from concourse.bass_utils import run_bass_kernel_spmd
from concourse.masks import make_identity

F32 = mybir.dt.float32
N = 2048
D = 1024
HD = 64
WIN = 256
NPAIR = 2  # head pairs per core (4 heads)
NDCH = D // 128  # 8
NKT = N // 128  # 16 key tiles / query tiles
SPAN = 384  # max query span per S^T key tile
QG = 512  # query-group size (QKV projection granularity)
NQG = N // QG  # 4
SCALE = 1.0 / np.sqrt(HD)

_MM_DT_NAME = os.environ.get("BASS_MM_DT", "bfloat16")
MM_DT = getattr(mybir.dt, _MM_DT_NAME)
_OUT_DT_NAME = os.environ.get("BASS_OUT_DT", "bfloat16")
OUT_DT = getattr(mybir.dt, _OUT_DT_NAME)

Exp = mybir.ActivationFunctionType.Exp
IS_GE = mybir.AluOpType.is_ge

# Pipeline grain: gg = one 256-query block (8 blocks).  QKV/v-nat run one
# block ahead (block gg+1 is projected during step gg), so S^T pieces
# (kt, lo, hi) can be emitted as soon as the needed qT columns exist;
# kt12/13 are split so only the structurally-last pieces come late.
S_EMIT = {
    0: [(0, 0, 384), (1, 0, 384)],
    1: [(2, 0, 384), (3, 0, 384)],
    2: [(4, 0, 384), (5, 0, 384)],
    3: [(6, 0, 384), (7, 0, 384)],
    4: [(8, 0, 384), (9, 0, 384)],
    5: [(10, 0, 384), (11, 0, 384), (12, 0, 256), (13, 0, 128)],
    6: [(12, 256, 384), (13, 128, 384), (14, 0, 256), (15, 0, 128)],
    7: [],
}
# query tiles whose AV inputs are complete at step gg
Q_EMIT = {0: [0, 1], 1: [2, 3], 2: [4, 5], 3: [6, 7], 4: [8, 9],
          5: [10, 11, 12, 13], 6: [14, 15], 7: []}


def _emit(ctx: ExitStack, tc: tile.TileContext, aps):
    nc = tc.nc
    xT, wq, wk, wv, wo, bq, bk, bv, out = aps
    MDT = MM_DT

    consts = ctx.enter_context(tc.tile_pool(name="consts", bufs=1))
    persist = ctx.enter_context(tc.tile_pool(name="persist", bufs=1))

    ident_f = consts.tile([128, 128], F32, tag="ident_f")
    make_identity(nc, ident_f)
    ident = consts.tile([128, 128], MDT, tag="ident")
    nc.vector.tensor_copy(ident, ident_f)

    # 0/1 band mask: valid iff r <= c <= r + WIN (keys on partitions,
    # query offset on free dim); duplicated on free dim for the 2-head tiles
    mask = consts.tile([128, SPAN], MDT, tag="mask")
    mask2 = consts.tile([128, 2, SPAN], MDT, tag="mask2")
    nc.gpsimd.memset(mask, 1.0)
    nc.gpsimd.affine_select(
        out=mask, in_=mask, compare_op=IS_GE, fill=0.0,
        base=0, pattern=[[1, SPAN]], channel_multiplier=-1,
    )  # keep c - r >= 0
    nc.gpsimd.affine_select(
        out=mask, in_=mask, compare_op=IS_GE, fill=0.0,
        base=WIN, pattern=[[-1, SPAN]], channel_multiplier=1,
    )  # keep r - c + WIN >= 0
    nc.gpsimd.tensor_copy(mask2[:, 0, :], mask)
    nc.gpsimd.tensor_copy(mask2[:, 1, :], mask)

    ones1 = consts.tile([1, 128], MDT, tag="ones1")
    nc.vector.memset(ones1, 1.0)
    onesf = consts.tile([128, 1], MDT, tag="onesf")
    nc.vector.memset(onesf, 1.0)

    # persistent intermediates
    qT = persist.tile([128, NPAIR, N], MDT, tag="qT")
    kT = persist.tile([128, NPAIR, N], MDT, tag="kT")
    vaug = persist.tile([128, NKT, NPAIR, 2, HD + 1], MDT, tag="vaug")
    U2 = persist.tile([128, NPAIR, N], MDT, tag="U2")
    bv_bc = persist.tile([128, 256], F32, tag="bv_bc")

    # pools
    xt_pool = ctx.enter_context(tc.tile_pool(name="xt", bufs=4))
    pt_pool = ctx.enter_context(tc.tile_pool(name="pt", bufs=12))
    un_pool = ctx.enter_context(tc.tile_pool(name="un", bufs=4))
    rc_pool = ctx.enter_context(tc.tile_pool(name="rc", bufs=4))
    ob_pool = ctx.enter_context(tc.tile_pool(name="ob", bufs=6))
    # PSUM: 8 banks x 2KB/partition total.
    #   psQ  [128,256] f32 x2 - q^T/k^T half-group accumulators
    #   ps512 [128,512] f32 x2 - v-nat pairs, out-proj, U-transpose, bias bcast
    #   psS  [128,384] f32 x2 - per-head S^T
    #   psAV [128,2,65] f32 x2 - transposed AV + denominators
    psQ = ctx.enter_context(tc.tile_pool(name="psQ", bufs=2, space="PSUM"))
    ps512 = ctx.enter_context(tc.tile_pool(name="ps512", bufs=2, space="PSUM"))
    psS = ctx.enter_context(tc.tile_pool(name="psS", bufs=2, space="PSUM"))
    psAV = ctx.enter_context(tc.tile_pool(name="psAV", bufs=2, space="PSUM"))

    # ---- input DMAs (contiguous 4KB descriptors on both sides) ------------
    xtg = {}
    for g in range(NQG):
        xtg[g] = xt_pool.tile([128, 2, NDCH, 256], MDT, tag="xtg",
                              name=f"xtg{g}")

    def dma_xtg_half(gg):
        g, half = gg // 2, gg % 2
        nc.sync.dma_start(out=xtg[g][:, half], in_=xT[:, gg])

    w_sb = {}
    for nm, wap in (("q", wq), ("k", wk), ("v", wv)):
        w_sb[nm] = consts.tile([128, NDCH, 256], MDT, tag=f"w{nm}", name=f"w{nm}")
    # first accumulation chunk rushed in tiny DMAs so compute starts early
    nc.sync.dma_start(out=w_sb["q"][:, 0, :], in_=wq[:, 0, :])
    nc.sync.dma_start(out=xtg[0][:, 0, 0, :], in_=xT[:, 0, 0, :])
    nc.sync.dma_start(out=w_sb["q"][:, 1:NDCH, :], in_=wq[:, 1:NDCH, :])
    nc.sync.dma_start(out=xtg[0][:, 0, 1:NDCH, :], in_=xT[:, 0, 1:NDCH, :])
    nc.sync.dma_start(out=w_sb["k"], in_=wk)
    b_sb = {}
    for nm, bap in (("q", bq), ("k", bk)):
        t = consts.tile([128, NPAIR], F32, tag=f"b{nm}", name=f"b{nm}")
        nc.sync.dma_start(out=t, in_=bap)
        b_sb[nm] = t
    nc.sync.dma_start(out=w_sb["v"], in_=wv)
    bv_sb = consts.tile([1, 256], MDT, tag="bv")
    nc.sync.dma_start(out=bv_sb, in_=bv)
    dma_xtg_half(1)
    wo_sb = consts.tile([128, NPAIR, D], MDT, tag="wo")
    nc.sync.dma_start(out=wo_sb, in_=wo)
    for gg in range(2, 8):
        dma_xtg_half(gg)

    # ones column of v_aug + bv broadcast (rank-1 matmul: ones^T x bv)
    nc.vector.tensor_copy(
        vaug[:, :, :, :, HD:HD + 1],
        onesf.broadcast_to((128, NKT, NPAIR, 2, 1)),
    )
    psb = ps512.tile([128, QG], F32, tag="ps512", name="psb")
    nc.tensor.matmul(psb[:, 0:256], ones1, bv_sb, start=True, stop=True)
    nc.vector.tensor_copy(bv_bc, psb[:, 0:256])

    # PE warm-up: the HAM clock gate holds the PE at 1.2 GHz until it sees
    # ~3.4us of sustained activity (and re-throttles after ~3.4us idle).
    # Burn the input-DMA wait on dummy matmuls so real work starts at 2.4 GHz.
    for i in range(64):
        wps = psQ.tile([128, 256], F32, tag="psQ", name="warm")
        nc.tensor.matmul(wps[:, 0:128], ident, ident, start=True, stop=True)

    # ---- helpers ----------------------------------------------------------
    pts = {}  # (kt, pair) -> pt tile

    def qkT(gg):
        g, half = gg // 2, gg % 2
        c0 = 256 * half
        for pair in range(NPAIR):
            for nm, dstT in (("q", qT), ("k", kT)):
                psq = psQ.tile([128, 256], F32, tag="psQ")
                for d in range(NDCH):
                    nc.tensor.matmul(
                        psq,
                        w_sb[nm][:, d, 128 * pair:128 * (pair + 1)],
                        xtg[g][:, half, d, :],
                        start=(d == 0), stop=(d == NDCH - 1),
                    )
                nc.vector.tensor_scalar_add(
                    dstT[:, pair, QG * g + c0:QG * g + c0 + 256], psq,
                    b_sb[nm][:, pair:pair + 1],
                )

    def vnat(gg):
        g, half = gg // 2, gg % 2
        psv = ps512.tile([128, QG], F32, tag="ps512", name="psv")
        for j in (0, 1):
            for d in range(NDCH):
                nc.tensor.matmul(
                    psv[:, 256 * j:256 * (j + 1)],
                    xtg[g][:, half, d, 128 * j:128 * (j + 1)],
                    w_sb["v"][:, d, :],
                    start=(d == 0), stop=(d == NDCH - 1),
                )
        for j in (0, 1):
            kt = 2 * gg + j
            nc.vector.tensor_add(
                vaug[:, kt, :, :, 0:HD],
                psv[:, 256 * j:256 * (j + 1)].rearrange(
                    "p (pair h d) -> p pair h d", pair=NPAIR, h=2
                ),
                bv_bc.rearrange(
                    "p (pair h d) -> p pair h d", pair=NPAIR, h=2
                ),
            )

    def s_piece(kt, lo, hi):
        for pair in range(NPAIR):
            if (kt, pair) in pts:
                pt = pts[(kt, pair)]
            else:
                pt = pt_pool.tile([128, 2, SPAN], MDT, tag="pt")
                pts[(kt, pair)] = pt
            for h in range(2):
                hb = 64 * h
                pss = psS.tile([128, SPAN], F32, tag="psS")
                nc.tensor.matmul(
                    pss[:, 0:hi - lo],
                    kT[hb:hb + 64, pair, 128 * kt:128 * kt + 128],
                    qT[hb:hb + 64, pair, 128 * kt + lo:128 * kt + hi],
                    start=True, stop=True,
                )
                nc.scalar.activation(
                    pt[:, h, lo:hi], pss[:, 0:hi - lo], Exp, scale=SCALE
                )
            m0, m1 = lo, min(hi, 128)
            if m0 < m1:
                nc.vector.tensor_mul(
                    pt[:, :, m0:m1], pt[:, :, m0:m1], mask2[:, :, m0:m1]
                )
            m0, m1 = max(lo, WIN), hi
            if m0 < m1:
                nc.vector.tensor_mul(
                    pt[:, :, m0:m1], pt[:, :, m0:m1], mask2[:, :, m0:m1]
                )

    def av(qt):
        kts = list(range(max(0, qt - 2), qt + 1))
        for pair in range(NPAIR):
            # one PSUM tile holds AV+denominators (f32 cols 0:130) and the
            # transposed normalized output (bf16, bitcast of cols 130:194)
            pst = psAV.tile([128, 194], F32, tag="psAV")
            psav = pst[:, 0:130].rearrange("p (h c) -> p h c", h=2)
            psu = pst[:, 130:194].bitcast(MDT)
            for h in range(2):
                for i, kt in enumerate(kts):
                    lo = 128 * (qt - kt)
                    nc.tensor.matmul(
                        psav[:, h, :],
                        pts[(kt, pair)][:, h, lo:lo + 128],
                        vaug[:, kt, pair, h, :],
                        start=(i == 0), stop=(i == len(kts) - 1),
                    )
            rc = rc_pool.tile([128, 2], F32, tag="rc")
            nc.vector.reciprocal_approx_fast(out=rc, in_=psav[:, :, HD])
            un = un_pool.tile([128, 2, HD], MDT, tag="un")
            for h in range(2):
                nc.vector.tensor_scalar_mul(
                    un[:, h, :], psav[:, h, 0:HD], rc[:, h:h + 1]
                )
            nc.tensor.transpose(
                psu, un.rearrange("p h d -> p (h d)"), ident
            )
            cp = nc.scalar.copy if (qt + pair) % 2 == 0 else nc.vector.tensor_copy
            cp(U2[:, pair, 128 * qt:128 * (qt + 1)], psu)

    _op_engines = [nc.vector.tensor_copy, nc.scalar.copy]

    def outproj(qt):
        ob = ob_pool.tile([128, D], OUT_DT, tag="ob")
        for dh in range(2):
            pso = ps512.tile([128, QG], F32, tag="ps512", name="pso")
            for pair in range(NPAIR):
                nc.tensor.matmul(
                    pso,
                    U2[:, pair, 128 * qt:128 * (qt + 1)],
                    wo_sb[:, pair, QG * dh:QG * (dh + 1)],
                    start=(pair == 0), stop=(pair == NPAIR - 1),
                )
            for c in (0, 1):  # halves copied on alternating engines
                cs = slice(256 * c, 256 * (c + 1))
                _op_engines[(qt + dh + c) % 2](
                    ob[:, QG * dh + 256 * c:QG * dh + 256 * (c + 1)],
                    pso[:, cs],
                )
        # one row-contiguous DMA per query tile (2KB descriptors)
        nc.sync.dma_start(out=out[128 * qt:128 * (qt + 1), :], in_=ob)

    # ---- pipeline ---------------------------------------------------------
    next_op = 0  # next query tile whose out-proj is pending

    def emit_ops(upto):
        nonlocal next_op
        while next_op <= upto:
            outproj(next_op)
            next_op += 1

    last_av = -1
    qkT(0)
    vnat(0)
    qkT(1)
    vnat(1)
    for gg in range(2 * NQG):
        if gg + 2 < 2 * NQG:  # project block gg+2 two steps ahead
            qkT(gg + 2)
            vnat(gg + 2)
        for kt, lo, hi in S_EMIT[gg]:
            s_piece(kt, lo, hi)
        emit_ops(last_av)  # catch up while exp/mask of new S pieces lands
        for qt in Q_EMIT[gg]:
            av(qt)
            last_av = qt
            emit_ops(qt - 1)
    emit_ops(NKT - 1)


def build():
    nc = bacc.Bacc("TRN2", target_bir_lowering=False, debug=False)
    xT = nc.dram_tensor(
        "xT", [128, 2 * NQG, NDCH, 256], MM_DT, kind="ExternalInput"
    ).ap()
    wq = nc.dram_tensor("wq", [128, NDCH, 256], MM_DT, kind="ExternalInput").ap()
    wk = nc.dram_tensor("wk", [128, NDCH, 256], MM_DT, kind="ExternalInput").ap()
    wv = nc.dram_tensor("wv", [128, NDCH, 256], MM_DT, kind="ExternalInput").ap()
    wo = nc.dram_tensor("wo", [128, NPAIR, D], MM_DT, kind="ExternalInput").ap()
    bq = nc.dram_tensor("bq", [128, NPAIR], F32, kind="ExternalInput").ap()
    bk = nc.dram_tensor("bk", [128, NPAIR], F32, kind="ExternalInput").ap()
    bv = nc.dram_tensor("bv", [1, 256], MM_DT, kind="ExternalInput").ap()
    out = nc.dram_tensor("out", [N, D], OUT_DT, kind="ExternalOutput").ap()
    with tile.TileContext(nc) as tc, ExitStack() as ctx:
        _emit(ctx, tc, (xT, wq, wk, wv, wo, bq, bk, bv, out))
    nc.compile()
    return nc


def shard_inputs(x, Wq, bq, Wk, bk, Wv, bv, Wo, bo):
    """Full inputs -> list of 8 per-core input maps (host-side layout prep)."""
    mdt = mybir.dt.np(MM_DT)

    def chunked(w):  # [1024, m] -> [128, 8, m]
        m = w.shape[1]
        return np.ascontiguousarray(
            w.reshape(NDCH, 128, m).transpose(1, 0, 2)
        ).astype(mdt)

    in_maps = []
    for c in range(8):
        b, hg = c // 4, c % 4
        cs = slice(256 * hg, 256 * (hg + 1))
        # xT[p, hh, d, c] = x[b][256*hh + c, 128*d + p]
        xt = np.ascontiguousarray(
            x[b].T.reshape(NDCH, 128, 2 * NQG, 256).transpose(1, 2, 0, 3)
        ).astype(mdt)
        in_maps.append({
            "xT": xt,
            "wq": chunked(Wq[:, cs]),
            "wk": chunked(Wk[:, cs]),
            "wv": chunked(Wv[:, cs]),
            "wo": np.ascontiguousarray(
                Wo[cs, :].reshape(NPAIR, 128, D).transpose(1, 0, 2)
            ).astype(mdt),
            "bq": np.ascontiguousarray(bq[cs].reshape(NPAIR, 128).T),
            "bk": np.ascontiguousarray(bk[cs].reshape(NPAIR, 128).T),
            "bv": np.ascontiguousarray(bv[cs].reshape(1, 256)).astype(mdt),
        })
    return in_maps


def assemble(results, bo):
    """8 per-core partial outputs -> full [2, N, D] output."""
    outs = [np.asarray(r["out"], dtype=np.float32) for r in results]
    full = np.empty((2, N, D), dtype=np.float32)
    for b in range(2):
        full[b] = outs[4 * b] + outs[4 * b + 1] + outs[4 * b + 2] + outs[4 * b + 3]
        full[b] += bo[None, :]
    return full


_NC_CACHE = {}


def _get_nc():
    key = (_MM_DT_NAME, _OUT_DT_NAME)
    if key not in _NC_CACHE:
        _NC_CACHE[key] = build()
    return _NC_CACHE[key]


def kernel(x, Wq, bq, Wk, bk, Wv, bv, Wo, bo, _trace=False):
    x, Wq, bq, Wk, bk, Wv, bv, Wo, bo = (
        np.asarray(a, dtype=np.float32)
        for a in (x, Wq, bq, Wk, bk, Wv, bv, Wo, bo)
    )
    nc = _get_nc()
    in_maps = shard_inputs(x, Wq, bq, Wk, bk, Wv, bv, Wo, bo)
    res = run_bass_kernel_spmd(nc, in_maps, core_ids=list(range(8)), trace=_trace)
    full = assemble(res.results, bo)
    if _trace:
        kernel.last_result = res
    return full
